# revision 14
# baseline (speedup 1.0000x reference)
"""InternLM3 custom attention on 8 TRN2 NeuronCores — v2 (bf16, pipelined).

Sharding: 4 heads per core (qk_w/v_w column-parallel by head); AllToAll in
two head-pair chunks converts the head-sharded attention output to
sequence-sharded; o-projection runs sequence-parallel with the full o_w
(bf16) resident in SBUF, prefetched during attention.

All matmul/vector operands are bf16 (fp32 PSUM accumulation), halving HBM
traffic and doubling DVE throughput vs fp32. K/V projection, RoPE and
attention are emitted interleaved per 512-column sequence quarter so PE
(matmuls), ACT (exp), DVE (rope/mask/copies) and DMA overlap. Attention is
computed transposed (S^T[k, q]); the softmax denominator rides as a ones
column appended to V; strictly-masked score columns are skipped at matmul
granularity and the 128-wide diagonal band is zeroed post-exp with one
triangular mask tile.
"""

import sys

sys.path.insert(0, "/opt/trn_rl_repo")

import numpy as np

import concourse.bass as bass
import concourse.tile as tile
from concourse import bacc, mybir
from concourse.bass import ds, ts
from concourse.bass_utils import run_bass_kernel_spmd

F32 = mybir.dt.float32
BF16 = mybir.dt.bfloat16
NCORES = 8
S = 2048          # sequence
HID = 2048        # hidden
NH = 32           # total heads
HD = 64           # head dim
HPC = NH // NCORES      # heads per core = 4
DPC = HPC * HD          # head-dims per core = 256
SSL = S // NCORES       # output seq slice per core = 256
VW = 66                 # V stride per head: 64 dims + 1 ones + 1 pad
ROPE_THETA = 10000.0


def build_program(sim_no_collective=False, repeat=1, debug_dump=False):
    nc = bacc.Bacc("TRN2", target_bir_lowering=False, debug=False,
                   num_devices=NCORES)

    # ---- I/O (all bf16 except the fp32 output) ----
    hidT = nc.dram_tensor("hidT", [HID, S], BF16, kind="ExternalInput").ap()
    qkwT = nc.dram_tensor("qkwT", [HID, DPC], BF16, kind="ExternalInput").ap()
    vwT = nc.dram_tensor("vwT", [HID, DPC], BF16, kind="ExternalInput").ap()
    owT = nc.dram_tensor("owT", [HID, HID], BF16, kind="ExternalInput").ap()
    xT_in = nc.dram_tensor("xT", [DPC, S], BF16, kind="ExternalInput").ap()
    xTs_in = nc.dram_tensor("xTs", [DPC, S], BF16, kind="ExternalInput").ap()
    cosT = nc.dram_tensor("cosT", [128, S], BF16, kind="ExternalInput").ap()
    sinT = nc.dram_tensor("sinT", [128, S], BF16, kind="ExternalInput").ap()
    triT = nc.dram_tensor("triT", [128, 128], BF16, kind="ExternalInput").ap()
    out_sl = nc.dram_tensor("out_slice", [SSL, HID], F32,
                            kind="ExternalOutput").ap()
    if debug_dump:
        dbg_att = nc.dram_tensor("dbg_att", [128, 2, S], F32,
                                 kind="ExternalOutput").ap()
        dbg_kt = nc.dram_tensor("dbg_kt", [128, 2, S], F32,
                                kind="ExternalOutput").ap()
        dbg_xt = nc.dram_tensor("dbg_xt", [128, 2, S], F32,
                                kind="ExternalOutput").ap()
        dbg_vt = nc.dram_tensor("dbg_vt", [128, 16, HPC, VW], F32,
                                kind="ExternalOutput").ap()

    def _emit(tc):
        for _rep in range(repeat):
            _emit_once(tc)

    def _emit_once(tc):
        with (
            nc.allow_low_precision(reason="bf16 operands, fp32 psum accum"),
            tc.tile_pool(name="const", bufs=1) as const,
            tc.tile_pool(name="dram", bufs=1, space="DRAM") as dram,
        ):
            # ---- persistent SBUF residents (DMAs emitted in the quarter
            # loop below so the first matmuls start as early as possible) ----
            qkw_t = const.tile([128, 16, DPC], BF16)
            qkwR = qkwT.rearrange("(n p) d -> p n d", p=128)
            vw_t = const.tile([128, 16, DPC], BF16)
            vwR = vwT.rearrange("(n p) d -> p n d", p=128)
            cos_t = const.tile([128, S], BF16)
            sin_t = const.tile([128, S], BF16)
            tri_t = const.tile([128, 128], BF16)
            xt = const.tile([128, 2, S], BF16)      # X^T, rope'd in place
            kt = const.tile([128, 2, S], BF16)      # K^T, rope'd in place
            v_t = const.tile([128, 16, HPC, VW], BF16)   # V + ones col
            att_t = const.tile([128, 2, S], BF16)   # attn^T assembled
            ones_t = const.tile([1, HD], BF16)

            # full o_w resident; prefetched in chunks on the scalar HWDGE
            # ring, spread across the quarters so it never head-blocks the
            # latency-critical sync-ring loads
            ow_t = const.tile([128, 16, HID], BF16)
            owR = owT.rearrange("(n p) d -> p n d", p=128)

            with (
                tc.tile_pool(name="hq", bufs=2) as hpool,
                tc.tile_pool(name="sw", bufs=4) as swp,
                tc.tile_pool(name="pp", bufs=6) as ppool,
                tc.tile_pool(name="rr", bufs=4) as rrp,
                tc.tile_pool(name="psk", bufs=2, space="PSUM") as psk,
                tc.tile_pool(name="psv", bufs=2, space="PSUM") as psv,
                tc.tile_pool(name="pss", bufs=2, space="PSUM") as pss,
                tc.tile_pool(name="pspv", bufs=1, space="PSUM") as pspv,
                tc.tile_pool(name="psbc", bufs=1, space="PSUM") as psbc,
            ):
                a2a_in = [dram.tile([NCORES, 128, SSL], BF16, name=f"a2ai{t}")
                          for t in range(2)]
                a2a_out = [dram.tile([NCORES * 128, SSL], BF16,
                                     name=f"a2ao{t}") for t in range(2)]

                def stage_collective(t):
                    for d in range(NCORES):
                        nc.scalar.dma_start(out=a2a_in[t][d],
                                            in_=att_t[:, t, ts(d, SSL)])
                    if sim_no_collective:
                        nc.sync.dma_start(
                            out=a2a_out[t][:],
                            in_=a2a_in[t][:].rearrange("d p s -> (d p) s"))
                    else:
                        nc.gpsimd.collective_compute(
                            "AllToAll",
                            mybir.AluOpType.bypass,
                            replica_groups=[list(range(NCORES))],
                            ins=[a2a_in[t][:].opt()],
                            outs=[a2a_out[t][:].opt()],
                        )

                hidR = hidT.rearrange("(n p) s -> p n s", p=128)

                def phase_a(sq):
                    pk = [psk.tile([128, 512], F32, tag='pk', name='pk')
                          for _ in range(2)]
                    hq = hpool.tile([128, 16, 512], BF16)
                    for c in range(4):
                        if sq == 0:
                            nc.sync.dma_start(out=qkw_t[:, ts(c, 4), :],
                                              in_=qkwR[:, ts(c, 4), :])
                        nc.sync.dma_start(out=hq[:, ts(c, 4), :],
                                          in_=hidR[:, ts(c, 4), ts(sq, 512)])
                        if sq == 0:
                            nc.sync.dma_start(out=vw_t[:, ts(c, 4), :],
                                              in_=vwR[:, ts(c, 4), :])
                    if sq == 0:
                        # remaining consts: queued behind the quarter-0
                        # operands, ahead of quarter 1
                        nc.sync.dma_start(out=tri_t[:], in_=triT)
                        nc.sync.dma_start(
                            out=xt[:],
                            in_=xT_in.rearrange("(t p) s -> p t s", p=128))
                        nc.sync.dma_start(out=cos_t[:], in_=cosT)
                        nc.sync.dma_start(out=sin_t[:], in_=sinT)
                        nc.vector.tensor_copy(out=ones_t[:],
                                              in_=tri_t[0:1, 64:128])
                        for st in range(16):
                            nc.vector.memset(v_t[:, st, :, HD:HD + 1], 1.0)
                    # V-projection runs as two passes of two seq-chunks so
                    # only two accumulation groups are live at once — a PSUM
                    # bank cannot host two groups (start= clears whole bank)
                    for half in range(2):
                        pv = [psv.tile([128, 256], F32, tag='pv', name='pv')
                              for _ in range(2)]
                        for hc in range(16):
                            if half == 0:
                                for m in range(2):
                                    nc.tensor.matmul(
                                        pk[m][:], qkw_t[:, hc, ts(m, 128)],
                                        hq[:, hc, :],
                                        start=(hc == 0), stop=(hc == 15))
                            for sub in range(2):
                                st4 = 2 * half + sub
                                nc.tensor.matmul(
                                    pv[sub][:],
                                    hq[:, hc, ts(st4, 128)], vw_t[:, hc, :],
                                    start=(hc == 0), stop=(hc == 15))
                        if half == 0:
                            for m in range(2):
                                nc.scalar.copy(out=kt[:, m, ts(sq, 512)],
                                               in_=pk[m][:])
                        for sub in range(2):
                            st4 = 2 * half + sub
                            for h in range(HPC):
                                nc.vector.tensor_copy(
                                    out=v_t[:, sq * 4 + st4, h, 0:HD],
                                    in_=pv[sub][:, ts(h, HD)])

                xTsR = xTs_in.rearrange("(t p) s -> p t s", p=128)

                def rope(sq):
                    q = ts(sq, 512)
                    # x: rotate-half swap comes precomputed from the host
                    xs = swp.tile([128, 2, 512], BF16, tag="sw")
                    nc.sync.dma_start(out=xs[:], in_=xTsR[:, :, q])
                    for t in range(2):
                        nc.vector.tensor_mul(out=xt[:, t, q], in0=xt[:, t, q],
                                             in1=cos_t[:, q])
                        nc.vector.tensor_mul(out=xs[:, t, :], in0=xs[:, t, :],
                                             in1=sin_t[:, q])
                        nc.vector.tensor_add(out=xt[:, t, q], in0=xt[:, t, q],
                                             in1=xs[:, t, :])
                    # k: swap rows 0:32 <-> 32:64 of each 64-row head block
                    ks = swp.tile([128, 2, 512], BF16, tag="sw")
                    for g in range(2):
                        b = 64 * g
                        nc.scalar.dma_start(out=ks[b:b + 32, :, :],
                                            in_=kt[b + 32:b + 64, :, q])
                        nc.scalar.dma_start(out=ks[b + 32:b + 64, :, :],
                                            in_=kt[b:b + 32, :, q])
                    for t in range(2):
                        nc.vector.tensor_mul(out=kt[:, t, q], in0=kt[:, t, q],
                                             in1=cos_t[:, q])
                        nc.vector.tensor_mul(out=ks[:, t, :], in0=ks[:, t, :],
                                             in1=sin_t[:, q])
                        nc.vector.tensor_add(out=kt[:, t, q], in0=kt[:, t, q],
                                             in1=ks[:, t, :])

                def attention(j):
                    q0 = 512 * j
                    nk = 4 * (j + 1)
                    for h in range(HPC):
                        hp = 64 * (h % 2)
                        htl = h // 2
                        pvp = pspv.tile([HD + 1, 512], F32, tag='pvp')
                        for i in range(nk):
                            r = 128 * i - q0
                            c0 = max(r, 0)   # cols < c0 are fully masked
                            sp = pss.tile([128, 512], F32, tag='sp')
                            nc.tensor.matmul(
                                sp[:, c0:512],
                                kt[hp:hp + HD, htl, ts(i, 128)],
                                xt[hp:hp + HD, htl, ds(q0 + c0, 512 - c0)],
                                start=True, stop=True)
                            pt = ppool.tile([128, 512], BF16, tag="pt")
                            nc.scalar.activation(
                                out=pt[:, c0:512], in_=sp[:, c0:512],
                                func=mybir.ActivationFunctionType.Exp,
                                scale=0.125)
                            if r >= 0:   # 128-wide diagonal band
                                nc.vector.tensor_mul(
                                    out=pt[:, c0:c0 + 128],
                                    in0=pt[:, c0:c0 + 128], in1=tri_t[:])
                            nc.tensor.matmul(
                                pvp[:, c0:512],
                                v_t[:, i, h, 0:HD + 1],
                                pt[:, c0:512],
                                start=(i == 0), stop=(i == nk - 1))
                        rec = rrp.tile([1, 512], BF16, tag="rec")
                        nc.vector.reciprocal(out=rec[:], in_=pvp[HD:HD + 1, :])
                        bc = psbc.tile([HD, 512], F32, tag='bc')
                        nc.tensor.matmul(bc[:], ones_t[:], rec[:],
                                         start=True, stop=True)
                        nc.scalar.copy(out=att_t[hp:hp + HD, htl, ds(q0, 512)],
                                       in_=pvp[0:HD, :])
                        nc.vector.tensor_mul(
                            out=att_t[hp:hp + HD, htl, ds(q0, 512)],
                            in0=att_t[hp:hp + HD, htl, ds(q0, 512)],
                            in1=bc[:])
                        if j == 3 and h == 1:
                            stage_collective(0)

                for sq in range(4):
                    phase_a(sq)
                    for oc in range(4):
                        hc = 4 * sq + oc
                        nc.scalar.dma_start(out=ow_t[:, hc, :],
                                            in_=owR[:, hc, :])
                    rope(sq)
                    attention(sq)
                stage_collective(1)
                if debug_dump:
                    cast = const.tile([128, 2, S], F32, name="dbgc")
                    nc.vector.tensor_copy(out=cast[:], in_=att_t[:])
                    nc.sync.dma_start(out=dbg_att, in_=cast[:])
                    nc.vector.tensor_copy(out=cast[:], in_=kt[:])
                    nc.sync.dma_start(out=dbg_kt, in_=cast[:])
                    nc.vector.tensor_copy(out=cast[:], in_=xt[:])
                    nc.sync.dma_start(out=dbg_xt, in_=cast[:])
                    castv = const.tile([128, 16, HPC, VW], F32, name="dbgv")
                    nc.vector.tensor_copy(out=castv[:], in_=v_t[:])
                    nc.sync.dma_start(out=dbg_vt, in_=castv[:])

            # =========== o-projection (sequence-parallel) ===========
            with (
                tc.tile_pool(name="af", bufs=2) as afp,
                tc.tile_pool(name="ob", bufs=1) as obp,
                tc.tile_pool(name="pso", bufs=8, space="PSUM") as pso,
            ):
                osb = obp.tile([128, 2, HID], F32)
                po = [[pso.tile([128, 512], F32, tag='po', name='po')
                       for tq in range(2)] for ob in range(4)]
                for t in range(2):
                    af = afp.tile([128, NCORES, SSL], BF16, tag="af")
                    nc.scalar.dma_start(
                        out=af[:],
                        in_=a2a_out[t][:].rearrange("(n p) s -> p n s", p=128))
                    for src in range(NCORES):
                        hc = 2 * src + t
                        for ob in range(4):
                            for tq in range(2):
                                nc.tensor.matmul(
                                    po[ob][tq][:],
                                    af[:, src, ts(tq, 128)],
                                    ow_t[:, hc, ts(ob, 512)],
                                    start=(t == 0 and src == 0),
                                    stop=(t == 1 and src == NCORES - 1))
                for ob in range(4):
                    for tq in range(2):
                        nc.scalar.copy(out=osb[:, tq, ts(ob, 512)],
                                       in_=po[ob][tq][:])
                for tq in range(2):
                    nc.scalar.dma_start(out=out_sl[ts(tq, 128), :],
                                        in_=osb[:, tq, :])

    with tile.TileContext(nc) as tc:
        _emit(tc)
    nc.compile()
    return nc


_PROGRAM = None


def _to_bf16(a):
    import ml_dtypes
    return np.asarray(a, dtype=np.float32).astype(ml_dtypes.bfloat16)


def _host_inputs(hidden_states, qk_w, v_w, o_w, position_ids):
    hs = np.asarray(hidden_states, dtype=np.float32)[0]          # [S, HID]
    qk_w = np.asarray(qk_w, dtype=np.float32)
    v_w = np.asarray(v_w, dtype=np.float32)
    o_w = np.asarray(o_w, dtype=np.float32)
    pos = np.asarray(position_ids)[0].astype(np.float64)         # [S]

    hidT = np.ascontiguousarray(hs.T)                            # [HID, S]
    owT = _to_bf16(np.ascontiguousarray(o_w.T))                  # [HID, HID]
    hidT16 = _to_bf16(hidT)

    inv_freq = 1.0 / (ROPE_THETA ** (np.arange(0, HD, 2, dtype=np.float64) / HD))
    freqs = pos[None, :] * inv_freq[:, None]                     # [32, S]
    emb = np.concatenate([freqs, freqs], axis=0)                 # [64, S]
    cos1 = np.cos(emb)
    sin_signed = np.sin(emb)
    sin_signed[:HD // 2] *= -1.0                                 # fold rotate sign
    cosT = _to_bf16(np.tile(cos1, (2, 1)))                       # [128, S]
    sinT = _to_bf16(np.tile(sin_signed, (2, 1)))

    kl = np.arange(128)[:, None]
    u = np.arange(128)[None, :]
    triT = _to_bf16((u >= kl).astype(np.float32))                # [128, 128]

    in_maps = []
    for c in range(NCORES):
        rows = slice(DPC * c, DPC * (c + 1))
        xT = hidT[rows]                                          # [256, S]
        xTs = np.empty_like(xT)                                  # rotate_half rows
        for h in range(HPC):
            b = HD * h
            xTs[b:b + 32] = xT[b + 32:b + 64]
            xTs[b + 32:b + 64] = xT[b:b + 32]
        in_maps.append({
            "hidT": hidT16,
            "qkwT": _to_bf16(np.ascontiguousarray(qk_w[rows].T)),
            "vwT": _to_bf16(np.ascontiguousarray(v_w[rows].T)),
            "owT": owT,
            "xT": _to_bf16(xT),
            "xTs": _to_bf16(xTs),
            "cosT": cosT,
            "sinT": sinT,
            "triT": triT,
        })
    return in_maps


def kernel(hidden_states, qk_w, v_w, o_w, position_ids, **extra):
    global _PROGRAM
    if _PROGRAM is None:
        _PROGRAM = build_program()
    in_maps = _host_inputs(hidden_states, qk_w, v_w, o_w, position_ids)
    res = run_bass_kernel_spmd(_PROGRAM, in_maps, list(range(NCORES)))
    out = np.concatenate([res.results[c]["out_slice"]
                          for c in range(NCORES)], axis=0)
    return out.reshape(1, S, HID).astype(np.float32)


# revision 15
# speedup vs baseline: 1.3615x; 1.3615x over previous
"""InternLM3 custom attention on 8 TRN2 NeuronCores — v2 (bf16, pipelined).

Sharding: 4 heads per core (qk_w/v_w column-parallel by head); AllToAll in
two head-pair chunks converts the head-sharded attention output to
sequence-sharded; o-projection runs sequence-parallel with the full o_w
(bf16) resident in SBUF, prefetched during attention.

All matmul/vector operands are bf16 (fp32 PSUM accumulation), halving HBM
traffic and doubling DVE throughput vs fp32. K/V projection, RoPE and
attention are emitted interleaved per 512-column sequence quarter so PE
(matmuls), ACT (exp), DVE (rope/mask/copies) and DMA overlap. Attention is
computed transposed (S^T[k, q]); the softmax denominator rides as a ones
column appended to V; strictly-masked score columns are skipped at matmul
granularity and the 128-wide diagonal band is zeroed post-exp with one
triangular mask tile.
"""

import sys

sys.path.insert(0, "/opt/trn_rl_repo")

import numpy as np

import concourse.bass as bass
import concourse.tile as tile
from concourse import bacc, mybir
from concourse.bass import ds, ts
from concourse.bass_utils import run_bass_kernel_spmd

F32 = mybir.dt.float32
BF16 = mybir.dt.bfloat16
NCORES = 8
S = 2048          # sequence
HID = 2048        # hidden
NH = 32           # total heads
HD = 64           # head dim
HPC = NH // NCORES      # heads per core = 4
DPC = HPC * HD          # head-dims per core = 256
SSL = S // NCORES       # output seq slice per core = 256
VW = 66                 # V stride per head: 64 dims + 1 ones + 1 pad
ROPE_THETA = 10000.0


def build_program(sim_no_collective=False, repeat=1, debug_dump=False):
    nc = bacc.Bacc("TRN2", target_bir_lowering=False, debug=False,
                   num_devices=NCORES)

    # ---- I/O (all bf16 except the fp32 output) ----
    hidT = nc.dram_tensor("hidT", [HID, S], BF16, kind="ExternalInput").ap()
    qkwT = nc.dram_tensor("qkwT", [HID, DPC], BF16, kind="ExternalInput").ap()
    vwT = nc.dram_tensor("vwT", [HID, DPC], BF16, kind="ExternalInput").ap()
    owT = nc.dram_tensor("owT", [HID, HID], BF16, kind="ExternalInput").ap()
    xT_in = nc.dram_tensor("xT", [DPC, S], BF16, kind="ExternalInput").ap()
    xTs_in = nc.dram_tensor("xTs", [DPC, S], BF16, kind="ExternalInput").ap()
    cosT = nc.dram_tensor("cosT", [128, S], BF16, kind="ExternalInput").ap()
    sinT = nc.dram_tensor("sinT", [128, S], BF16, kind="ExternalInput").ap()
    triT = nc.dram_tensor("triT", [128, 128], BF16, kind="ExternalInput").ap()
    out_sl = nc.dram_tensor("out_slice", [SSL, HID], F32,
                            kind="ExternalOutput").ap()
    if debug_dump:
        dbg_att = nc.dram_tensor("dbg_att", [128, 2, S], F32,
                                 kind="ExternalOutput").ap()
        dbg_kt = nc.dram_tensor("dbg_kt", [128, 2, S], F32,
                                kind="ExternalOutput").ap()
        dbg_xt = nc.dram_tensor("dbg_xt", [128, 2, S], F32,
                                kind="ExternalOutput").ap()
        dbg_vt = nc.dram_tensor("dbg_vt", [128, 16, HPC, VW], F32,
                                kind="ExternalOutput").ap()

    def _emit(tc):
        for _rep in range(repeat):
            _emit_once(tc)

    def _emit_once(tc):
        with (
            nc.allow_low_precision(reason="bf16 operands, fp32 psum accum"),
            tc.tile_pool(name="const", bufs=1) as const,
            tc.tile_pool(name="dram", bufs=1, space="DRAM") as dram,
        ):
            # ---- persistent SBUF residents (DMAs emitted in the quarter
            # loop below so the first matmuls start as early as possible) ----
            qkw_t = const.tile([128, 16, DPC], BF16)
            qkwR = qkwT.rearrange("(n p) d -> p n d", p=128)
            vw_t = const.tile([128, 16, DPC], BF16)
            vwR = vwT.rearrange("(n p) d -> p n d", p=128)
            cos_t = const.tile([128, S], BF16)
            sin_t = const.tile([128, S], BF16)
            tri_t = const.tile([128, 128], BF16)
            xt = const.tile([128, 2, S], BF16)      # X^T, rope'd in place
            kt = const.tile([128, 2, S], BF16)      # K^T, rope'd in place
            v_t = const.tile([128, 16, HPC, VW], BF16)   # V + ones col
            att_t = const.tile([128, 2, S], BF16)   # attn^T assembled
            ones_t = const.tile([1, HD], BF16)

            # full o_w resident; prefetched in chunks on the scalar HWDGE
            # ring, spread across the quarters so it never head-blocks the
            # latency-critical sync-ring loads
            ow_t = const.tile([128, 16, HID], BF16)
            owR = owT.rearrange("(n p) d -> p n d", p=128)

            with (
                tc.tile_pool(name="hq", bufs=2) as hpool,
                tc.tile_pool(name="sw", bufs=4) as swp,
                tc.tile_pool(name="pp", bufs=6) as ppool,
                tc.tile_pool(name="rr", bufs=4) as rrp,
                tc.tile_pool(name="psk", bufs=2, space="PSUM") as psk,
                tc.tile_pool(name="psv", bufs=2, space="PSUM") as psv,
                tc.tile_pool(name="pss", bufs=2, space="PSUM") as pss,
                tc.tile_pool(name="pspv", bufs=1, space="PSUM") as pspv,
                tc.tile_pool(name="psbc", bufs=1, space="PSUM") as psbc,
            ):
                a2a_in = [dram.tile([NCORES, 128, SSL], BF16, name=f"a2ai{t}")
                          for t in range(2)]
                a2a_out = [dram.tile([NCORES * 128, SSL], BF16,
                                     name=f"a2ao{t}") for t in range(2)]

                def stage_collective(t):
                    for d in range(NCORES):
                        nc.scalar.dma_start(out=a2a_in[t][d],
                                            in_=att_t[:, t, ts(d, SSL)])
                    if sim_no_collective:
                        nc.sync.dma_start(
                            out=a2a_out[t][:],
                            in_=a2a_in[t][:].rearrange("d p s -> (d p) s"))
                    else:
                        nc.gpsimd.collective_compute(
                            "AllToAll",
                            mybir.AluOpType.bypass,
                            replica_groups=[list(range(NCORES))],
                            ins=[a2a_in[t][:].opt()],
                            outs=[a2a_out[t][:].opt()],
                        )

                hidR = hidT.rearrange("(n p) s -> p n s", p=128)

                def phase_a(sq):
                    pk = [psk.tile([128, 512], F32, tag='pk', name='pk')
                          for _ in range(2)]
                    hq = hpool.tile([128, 16, 512], BF16)
                    for c in range(4):
                        if sq == 0:
                            nc.sync.dma_start(out=qkw_t[:, ts(c, 4), :],
                                              in_=qkwR[:, ts(c, 4), :])
                        nc.sync.dma_start(out=hq[:, ts(c, 4), :],
                                          in_=hidR[:, ts(c, 4), ts(sq, 512)])
                        if sq == 0:
                            nc.sync.dma_start(out=vw_t[:, ts(c, 4), :],
                                              in_=vwR[:, ts(c, 4), :])
                    if sq == 0:
                        # remaining consts: queued behind the quarter-0
                        # operands, ahead of quarter 1
                        nc.sync.dma_start(out=tri_t[:], in_=triT)
                        nc.sync.dma_start(
                            out=xt[:],
                            in_=xT_in.rearrange("(t p) s -> p t s", p=128))
                        nc.sync.dma_start(out=cos_t[:], in_=cosT)
                        nc.sync.dma_start(out=sin_t[:], in_=sinT)
                        nc.vector.tensor_copy(out=ones_t[:],
                                              in_=tri_t[0:1, 64:128])
                        for st in range(16):
                            nc.vector.memset(v_t[:, st, :, HD:HD + 1], 1.0)
                    # V-projection runs as two passes of two seq-chunks so
                    # only two accumulation groups are live at once — a PSUM
                    # bank cannot host two groups (start= clears whole bank)
                    for half in range(2):
                        pv = [psv.tile([128, 256], F32, tag='pv', name='pv')
                              for _ in range(2)]
                        for hc in range(16):
                            if half == 0:
                                for m in range(2):
                                    nc.tensor.matmul(
                                        pk[m][:], qkw_t[:, hc, ts(m, 128)],
                                        hq[:, hc, :],
                                        start=(hc == 0), stop=(hc == 15))
                            for sub in range(2):
                                st4 = 2 * half + sub
                                nc.tensor.matmul(
                                    pv[sub][:],
                                    hq[:, hc, ts(st4, 128)], vw_t[:, hc, :],
                                    start=(hc == 0), stop=(hc == 15))
                        if half == 0:
                            for m in range(2):
                                nc.vector.tensor_copy(out=kt[:, m, ts(sq, 512)],
                                                      in_=pk[m][:])
                        for sub in range(2):
                            st4 = 2 * half + sub
                            for h in range(HPC):
                                nc.vector.tensor_copy(
                                    out=v_t[:, sq * 4 + st4, h, 0:HD],
                                    in_=pv[sub][:, ts(h, HD)])

                xTsR = xTs_in.rearrange("(t p) s -> p t s", p=128)

                def rope(sq):
                    q = ts(sq, 512)
                    # x: rotate-half swap comes precomputed from the host
                    xs = swp.tile([128, 2, 512], BF16, tag="sw")
                    nc.sync.dma_start(out=xs[:], in_=xTsR[:, :, q])
                    for t in range(2):
                        nc.vector.tensor_mul(out=xt[:, t, q], in0=xt[:, t, q],
                                             in1=cos_t[:, q])
                        nc.vector.tensor_mul(out=xs[:, t, :], in0=xs[:, t, :],
                                             in1=sin_t[:, q])
                        nc.vector.tensor_add(out=xt[:, t, q], in0=xt[:, t, q],
                                             in1=xs[:, t, :])
                    # k: swap rows 0:32 <-> 32:64 of each 64-row head block
                    ks = swp.tile([128, 2, 512], BF16, tag="sw")
                    for g in range(2):
                        b = 64 * g
                        nc.scalar.dma_start(out=ks[b:b + 32, :, :],
                                            in_=kt[b + 32:b + 64, :, q])
                        nc.scalar.dma_start(out=ks[b + 32:b + 64, :, :],
                                            in_=kt[b:b + 32, :, q])
                    for t in range(2):
                        nc.vector.tensor_mul(out=kt[:, t, q], in0=kt[:, t, q],
                                             in1=cos_t[:, q])
                        nc.vector.tensor_mul(out=ks[:, t, :], in0=ks[:, t, :],
                                             in1=sin_t[:, q])
                        nc.vector.tensor_add(out=kt[:, t, q], in0=kt[:, t, q],
                                             in1=ks[:, t, :])

                def attention(j):
                    q0 = 512 * j
                    nk = 4 * (j + 1)
                    for h in range(HPC):
                        hp = 64 * (h % 2)
                        htl = h // 2
                        pvp = pspv.tile([HD + 1, 512], F32, tag='pvp')
                        for i in range(nk):
                            r = 128 * i - q0
                            c0 = max(r, 0)   # cols < c0 are fully masked
                            sp = pss.tile([128, 512], F32, tag='sp')
                            nc.tensor.matmul(
                                sp[:, c0:512],
                                kt[hp:hp + HD, htl, ts(i, 128)],
                                xt[hp:hp + HD, htl, ds(q0 + c0, 512 - c0)],
                                start=True, stop=True)
                            pt = ppool.tile([128, 512], BF16, tag="pt")
                            nc.scalar.activation(
                                out=pt[:, c0:512], in_=sp[:, c0:512],
                                func=mybir.ActivationFunctionType.Exp,
                                scale=0.125)
                            if r >= 0:   # 128-wide diagonal band
                                nc.vector.tensor_mul(
                                    out=pt[:, c0:c0 + 128],
                                    in0=pt[:, c0:c0 + 128], in1=tri_t[:])
                            nc.tensor.matmul(
                                pvp[:, c0:512],
                                v_t[:, i, h, 0:HD + 1],
                                pt[:, c0:512],
                                start=(i == 0), stop=(i == nk - 1))
                        rec = rrp.tile([1, 512], BF16, tag="rec")
                        nc.vector.reciprocal(out=rec[:], in_=pvp[HD:HD + 1, :])
                        bc = psbc.tile([HD, 512], F32, tag='bc')
                        nc.tensor.matmul(bc[:], ones_t[:], rec[:],
                                         start=True, stop=True)
                        nc.vector.tensor_copy(
                            out=att_t[hp:hp + HD, htl, ds(q0, 512)],
                            in_=pvp[0:HD, :])
                        nc.vector.tensor_mul(
                            out=att_t[hp:hp + HD, htl, ds(q0, 512)],
                            in0=att_t[hp:hp + HD, htl, ds(q0, 512)],
                            in1=bc[:])
                        if j == 3 and h == 1:
                            stage_collective(0)

                for sq in range(4):
                    phase_a(sq)
                    for oc in range(4):
                        hc = 4 * sq + oc
                        nc.scalar.dma_start(out=ow_t[:, hc, :],
                                            in_=owR[:, hc, :])
                    rope(sq)
                    attention(sq)
                stage_collective(1)
                if debug_dump:
                    cast = const.tile([128, 2, S], F32, name="dbgc")
                    nc.vector.tensor_copy(out=cast[:], in_=att_t[:])
                    nc.sync.dma_start(out=dbg_att, in_=cast[:])
                    nc.vector.tensor_copy(out=cast[:], in_=kt[:])
                    nc.sync.dma_start(out=dbg_kt, in_=cast[:])
                    nc.vector.tensor_copy(out=cast[:], in_=xt[:])
                    nc.sync.dma_start(out=dbg_xt, in_=cast[:])
                    castv = const.tile([128, 16, HPC, VW], F32, name="dbgv")
                    nc.vector.tensor_copy(out=castv[:], in_=v_t[:])
                    nc.sync.dma_start(out=dbg_vt, in_=castv[:])

            # =========== o-projection (sequence-parallel) ===========
            with (
                tc.tile_pool(name="af", bufs=2) as afp,
                tc.tile_pool(name="ob", bufs=1) as obp,
                tc.tile_pool(name="pso", bufs=8, space="PSUM") as pso,
            ):
                osb = obp.tile([128, 2, HID], F32)
                po = [[pso.tile([128, 512], F32, tag='po', name='po')
                       for tq in range(2)] for ob in range(4)]
                for t in range(2):
                    af = afp.tile([128, NCORES, SSL], BF16, tag="af")
                    nc.scalar.dma_start(
                        out=af[:],
                        in_=a2a_out[t][:].rearrange("(n p) s -> p n s", p=128))
                    for src in range(NCORES):
                        hc = 2 * src + t
                        for ob in range(4):
                            for tq in range(2):
                                nc.tensor.matmul(
                                    po[ob][tq][:],
                                    af[:, src, ts(tq, 128)],
                                    ow_t[:, hc, ts(ob, 512)],
                                    start=(t == 0 and src == 0),
                                    stop=(t == 1 and src == NCORES - 1))
                for ob in range(4):
                    for tq in range(2):
                        nc.scalar.copy(out=osb[:, tq, ts(ob, 512)],
                                       in_=po[ob][tq][:])
                for tq in range(2):
                    nc.scalar.dma_start(out=out_sl[ts(tq, 128), :],
                                        in_=osb[:, tq, :])

    with tile.TileContext(nc) as tc:
        _emit(tc)
    nc.compile()
    return nc


_PROGRAM = None


def _to_bf16(a):
    import ml_dtypes
    return np.asarray(a, dtype=np.float32).astype(ml_dtypes.bfloat16)


def _host_inputs(hidden_states, qk_w, v_w, o_w, position_ids):
    hs = np.asarray(hidden_states, dtype=np.float32)[0]          # [S, HID]
    qk_w = np.asarray(qk_w, dtype=np.float32)
    v_w = np.asarray(v_w, dtype=np.float32)
    o_w = np.asarray(o_w, dtype=np.float32)
    pos = np.asarray(position_ids)[0].astype(np.float64)         # [S]

    hidT = np.ascontiguousarray(hs.T)                            # [HID, S]
    owT = _to_bf16(np.ascontiguousarray(o_w.T))                  # [HID, HID]
    hidT16 = _to_bf16(hidT)

    inv_freq = 1.0 / (ROPE_THETA ** (np.arange(0, HD, 2, dtype=np.float64) / HD))
    freqs = pos[None, :] * inv_freq[:, None]                     # [32, S]
    emb = np.concatenate([freqs, freqs], axis=0)                 # [64, S]
    cos1 = np.cos(emb)
    sin_signed = np.sin(emb)
    sin_signed[:HD // 2] *= -1.0                                 # fold rotate sign
    cosT = _to_bf16(np.tile(cos1, (2, 1)))                       # [128, S]
    sinT = _to_bf16(np.tile(sin_signed, (2, 1)))

    kl = np.arange(128)[:, None]
    u = np.arange(128)[None, :]
    triT = _to_bf16((u >= kl).astype(np.float32))                # [128, 128]

    in_maps = []
    for c in range(NCORES):
        rows = slice(DPC * c, DPC * (c + 1))
        xT = hidT[rows]                                          # [256, S]
        xTs = np.empty_like(xT)                                  # rotate_half rows
        for h in range(HPC):
            b = HD * h
            xTs[b:b + 32] = xT[b + 32:b + 64]
            xTs[b + 32:b + 64] = xT[b:b + 32]
        in_maps.append({
            "hidT": hidT16,
            "qkwT": _to_bf16(np.ascontiguousarray(qk_w[rows].T)),
            "vwT": _to_bf16(np.ascontiguousarray(v_w[rows].T)),
            "owT": owT,
            "xT": _to_bf16(xT),
            "xTs": _to_bf16(xTs),
            "cosT": cosT,
            "sinT": sinT,
            "triT": triT,
        })
    return in_maps


def kernel(hidden_states, qk_w, v_w, o_w, position_ids, **extra):
    global _PROGRAM
    if _PROGRAM is None:
        _PROGRAM = build_program()
    in_maps = _host_inputs(hidden_states, qk_w, v_w, o_w, position_ids)
    res = run_bass_kernel_spmd(_PROGRAM, in_maps, list(range(NCORES)))
    out = np.concatenate([res.results[c]["out_slice"]
                          for c in range(NCORES)], axis=0)
    return out.reshape(1, S, HID).astype(np.float32)
